# revision 39
# baseline (speedup 1.0000x reference)
"""DistMult decoder kernel for Trainium2 (Bass/Tile), 8-core data-parallel.

Computes sigmoid(einsum('nd,d,nd->n', row, rel, col)) for N=500000, D=256.

Sharding: rows split evenly across 8 cores (62500 each). The selected
relation vector rel = relations[relation_index] is broadcast to [128, 256]
on host (tiny) and replicated to every core.

Per-core layout: the 62500-row shard is viewed as [128, 488, 256]
(n = p*488 + j) plus a 36-row tail, so every DMA moves per-partition
contiguous spans (16 KB for CHUNK=16). Engines:
  - HWDGE (sync + scalar rings): input loads
  - GPSIMD: prod = row * col   (elementwise)
  - DVE: tensor_tensor_reduce: scores[p, j] = sum_d(prod * rel)
  - ACT: sigmoid
"""

import numpy as np

import concourse.bass as bass
import concourse.mybir as mybir
from concourse import tile
from concourse.bass_utils import run_bass_kernel_spmd

N = 500000
D = 256
N_CORES = 8
N_SHARD = N // N_CORES  # 62500
P = 128
J = N_SHARD // P        # 488
MAIN = P * J            # 62464
TAIL = N_SHARD - MAIN   # 36
CHUNK = 16              # j-columns per DMA chunk (16 KB contiguous/partition)

F32 = mybir.dt.float32


def build_program(n_shard: int = N_SHARD, chunk: int = CHUNK, bufs: int = 5) -> bass.Bass:
    p = P
    j_cols = n_shard // p
    main = p * j_cols
    tail = n_shard - main

    nc = bass.Bass()
    row = nc.declare_dram_parameter("row", [n_shard, D], F32, isOutput=False)
    col = nc.declare_dram_parameter("col", [n_shard, D], F32, isOutput=False)
    relb = nc.declare_dram_parameter("relb", [p, D], F32, isOutput=False)
    out = nc.declare_dram_parameter("out", [n_shard], F32, isOutput=True)

    row_m = row[0:main, :].rearrange("(p j) d -> p j d", p=p)
    col_m = col[0:main, :].rearrange("(p j) d -> p j d", p=p)
    out_m = out[0:main].rearrange("(p j) -> p j", p=p)

    mult = mybir.AluOpType.mult
    add = mybir.AluOpType.add
    sig = mybir.ActivationFunctionType.Sigmoid

    # chunk schedule over the main part: small ramp-up chunks first so the
    # DVE starts sooner, then full-size chunks
    sizes = []
    left = j_cols
    for r in (max(1, chunk // 4), max(1, chunk // 2)):
        if left > chunk:
            sizes.append(min(r, left))
            left -= sizes[-1]
    while left > 0:
        sizes.append(min(chunk, left))
        left -= sizes[-1]
    n_chunks = len(sizes)

    from contextlib import ExitStack

    with ExitStack() as es:
        rel_sb = es.enter_context(nc.sbuf_tensor([p, D], F32))
        scores = es.enter_context(nc.sbuf_tensor([p, j_cols + 1], F32))
        rt_buf = es.enter_context(nc.sbuf_tensor([p, bufs * chunk * D], F32))
        ct_buf = es.enter_context(nc.sbuf_tensor([p, bufs * chunk * D], F32))
        rt_t = es.enter_context(nc.sbuf_tensor([p, D], F32))
        ct_t = es.enter_context(nc.sbuf_tensor([p, D], F32))
        rel_sem = es.enter_context(nc.semaphore("rel_sem"))
        row_sems = [
            es.enter_context(nc.semaphore(f"row_sem{b}")) for b in range(bufs)
        ]
        col_sems = [
            es.enter_context(nc.semaphore(f"col_sem{b}")) for b in range(bufs)
        ]
        tt_sems = [
            es.enter_context(nc.semaphore(f"tt_sem{b}")) for b in range(bufs)
        ]
        tt2_sems = [
            es.enter_context(nc.semaphore(f"tt2_sem{b}")) for b in range(bufs)
        ]
        tail_sem = es.enter_context(nc.semaphore("tail_sem"))
        tail_dve_sem = es.enter_context(nc.semaphore("tail_dve_sem"))
        ve_sem = es.enter_context(nc.semaphore("ve_sem"))
        act_sem = es.enter_context(nc.semaphore("act_sem"))
        store_sem = es.enter_context(nc.semaphore("store_sem"))
        block = es.enter_context(nc.Block())

        def rt_slot(c):
            b = c % bufs
            return rt_buf[:, b * chunk * D : (b + 1) * chunk * D]

        def ct_slot(c):
            b = c % bufs
            return ct_buf[:, b * chunk * D : (b + 1) * chunk * D]

        # ve_sem counts STT completions only. Waiting for the cumulative
        # total of all incs issued so far guarantees completion of every one
        # of them, even with unordered retirement. TT completions tick the
        # per-slot tt_sems (one TT in flight per slot at a time, so a
        # cumulative per-slot count is also sound).
        cum_stt = []
        tot = 0
        for k in sizes:
            tot += k
            cum_stt.append(tot)
        stt_total = tot

        # chunk start offsets
        offs = []
        o = 0
        for k in sizes:
            offs.append(o)
            o += k

        # number of store groups (>=128 j-columns each); must match the
        # scalar block's grouping below
        n_store_groups = 0
        acc = 0
        for c, k in enumerate(sizes):
            acc += k
            if acc >= 128 or c == n_chunks - 1:
                n_store_groups += 1
                acc = 0

        @block.sync
        def _(sync):
            col_lead = min(2, bufs - 1)

            def issue_col(c):
                k = sizes[c]
                j0 = offs[c]
                if c >= bufs:
                    # slot reuse: chunk c-bufs must be fully consumed (STTs
                    # are the last readers of both the rt and ct slabs)
                    sync.wait_ge(ve_sem, cum_stt[c - bufs])
                sync.dma_start(
                    ct_slot(c)[:, 0 : k * D], col_m[:, j0 : j0 + k, :]
                ).then_inc(col_sems[c % bufs], 16)

            def issue_row(c):
                k = sizes[c]
                j0 = offs[c]
                # ct-slot gate for chunk c already passed (issued with or
                # before this row's stronger col gate), rt shares thresholds
                sync.dma_start(
                    rt_slot(c)[:, 0 : k * D], row_m[:, j0 : j0 + k, :]
                ).then_inc(row_sems[c % bufs], 16)

            sync.dma_start(rel_sb[:], relb[:]).then_inc(rel_sem, 16)
            # cols run col_lead chunks ahead of rows in the FIFO, matching
            # the DVE's TT lookahead
            for c0 in range(min(col_lead, n_chunks)):
                issue_col(c0)
            for c in range(n_chunks):
                if c + col_lead < n_chunks:
                    issue_col(c + col_lead)
                issue_row(c)
            if tail:
                sync.dma_start(rt_t[0:tail, :], row[main:n_shard, :]).then_inc(
                    tail_sem, 16
                )
                sync.dma_start(ct_t[0:tail, :], col[main:n_shard, :]).then_inc(
                    tail_sem, 16
                )
            # all stores issued from the scalar engine; just await completion
            sync.wait_ge(store_sem, 16 * (n_store_groups + (1 if tail else 0)))

        @block.vector
        def _(vector):
            def do_tt(c):
                k = sizes[c]
                vector.wait_ge(col_sems[c % bufs], 16 * (c // bufs + 1))
                ct_v = ct_slot(c)[:, 0 : k * D].rearrange("p (k d) -> p k d", d=D)
                vector.tensor_tensor(
                    out=ct_v,
                    in0=ct_v,
                    in1=rel_sb[:].unsqueeze(1).broadcast_to([p, k, D]),
                    op=mult,
                ).then_inc(tt_sems[c % bufs], 1)

            lookahead = min(2, bufs - 1)
            vector.wait_ge(rel_sem, 16)
            for c0 in range(min(lookahead, n_chunks)):
                do_tt(c0)
            for c, k in enumerate(sizes):
                if c + lookahead < n_chunks:
                    # lookahead: issue later chunks' TT1s before this chunk's
                    # TT2 so the tt_sem wait below is already satisfied
                    do_tt(c + lookahead)
                vector.wait_ge(tt_sems[c % bufs], c // bufs + 1)
                vector.wait_ge(row_sems[c % bufs], 16 * (c // bufs + 1))
                # ct *= rt; the ACT engine then reduces each j-slab
                vector.tensor_tensor(
                    out=ct_slot(c)[:, 0 : k * D],
                    in0=ct_slot(c)[:, 0 : k * D],
                    in1=rt_slot(c)[:, 0 : k * D],
                    op=mult,
                ).then_inc(tt2_sems[c % bufs], 1)
            if tail:
                vector.wait_ge(tail_sem, 32)
                vector.tensor_tensor(
                    out=ct_t[0:tail, :],
                    in0=ct_t[0:tail, :],
                    in1=rel_sb[0:tail, 0:D],
                    op=mult,
                ).then_inc(tail_dve_sem, 1)
                vector.wait_ge(tail_dve_sem, 1)
                vector.tensor_tensor(
                    out=ct_t[0:tail, :],
                    in0=ct_t[0:tail, :],
                    in1=rt_t[0:tail, :],
                    op=mult,
                ).then_inc(tail_dve_sem, 1)

        # store groups: batch chunks until >=128 j-columns so each store DMA
        # moves >=512 B per partition (avoids the sub-512B RMW penalty)
        groups = []  # (last_chunk_idx, g0, g1)
        g0 = 0
        acc = 0
        for c, k in enumerate(sizes):
            acc += k
            if acc >= 128 or c == n_chunks - 1:
                groups.append((c, g0, g0 + acc))
                g0 += acc
                acc = 0
        n_groups = len(groups)

        cp = mybir.ActivationFunctionType.Copy

        @block.scalar
        def _(scalar):
            # ACT owns the reduction pass: per j-slab Copy-with-accumulate
            # gives scores[:, j] = sum_d(ct_slab). Then grouped sigmoid +
            # store on the ACT HWDGE ring.
            gi = 0
            for c, k in enumerate(sizes):
                j0 = offs[c]
                scalar.wait_ge(tt2_sems[c % bufs], c // bufs + 1)
                for jj in range(k):
                    sl = ct_slot(c)[:, jj * D : (jj + 1) * D]
                    scalar.activation(
                        out=sl,
                        in_=sl,
                        func=cp,
                        accum_out=scores[:, j0 + jj : j0 + jj + 1],
                    ).then_inc(ve_sem, 1)
                if gi < n_groups and groups[gi][0] == c:
                    _, a, b = groups[gi]
                    scalar.wait_ge(ve_sem, cum_stt[c])
                    scalar.activation(
                        out=scores[:, a:b], in_=scores[:, a:b], func=sig
                    ).then_inc(act_sem, 1)
                    scalar.wait_ge(act_sem, gi + 1)
                    scalar.dma_start(out_m[:, a:b], scores[:, a:b]).then_inc(
                        store_sem, 16
                    )
                    gi += 1
            if tail:
                scalar.wait_ge(tail_dve_sem, 2)
                scalar.activation(
                    out=ct_t[0:tail, :],
                    in_=ct_t[0:tail, :],
                    func=cp,
                    accum_out=scores[0:tail, j_cols : j_cols + 1],
                ).then_inc(ve_sem, 1)
                scalar.wait_ge(ve_sem, stt_total + 1)
                scalar.activation(
                    out=scores[0:tail, j_cols : j_cols + 1],
                    in_=scores[0:tail, j_cols : j_cols + 1],
                    func=sig,
                ).then_inc(act_sem, 1)
                scalar.wait_ge(act_sem, n_groups + 1)
                with nc.allow_non_contiguous_dma(reason="tiny tail store"):
                    scalar.dma_start(
                        out[main:n_shard].rearrange("(p j) -> p j", j=1),
                        scores[0:tail, j_cols : j_cols + 1],
                    ).then_inc(store_sem, 16)

    return nc


_PROGRAM = None


def _get_program() -> bass.Bass:
    global _PROGRAM
    if _PROGRAM is None:
        _PROGRAM = build_program()
    return _PROGRAM


def make_relb(rel, chunk=CHUNK):
    """Host-side [P, D] broadcast of the selected relation vector."""
    del chunk
    return np.ascontiguousarray(
        np.broadcast_to(np.asarray(rel, np.float32), (P, D))
    )


def _run(inputs_row, inputs_col, relations, relation_index, **spmd_kwargs):
    inputs_row = np.ascontiguousarray(np.asarray(inputs_row, dtype=np.float32))
    inputs_col = np.ascontiguousarray(np.asarray(inputs_col, dtype=np.float32))
    relations = np.asarray(relations, dtype=np.float32)
    idx = int(relation_index)

    relb = make_relb(relations[idx])

    in_maps = []
    for c in range(N_CORES):
        sl = slice(c * N_SHARD, (c + 1) * N_SHARD)
        in_maps.append(
            {
                "row": inputs_row[sl],
                "col": inputs_col[sl],
                "relb": relb,
            }
        )

    nc = _get_program()
    return run_bass_kernel_spmd(nc, in_maps, list(range(N_CORES)), **spmd_kwargs)


def kernel(inputs_row, inputs_col, relations, relation_index):
    results = _run(inputs_row, inputs_col, relations, relation_index).results
    out = np.concatenate([results[c]["out"] for c in range(N_CORES)])
    return out.astype(np.float32, copy=False)


if __name__ == "__main__":
    rng = np.random.default_rng(0)
    inputs = {
        "inputs_row": rng.standard_normal((N, D), dtype=np.float32),
        "inputs_col": rng.standard_normal((N, D), dtype=np.float32),
        "relations": rng.standard_normal((8, D), dtype=np.float32),
        "relation_index": 3,
    }
    got = kernel(**inputs)
    rel = inputs["relations"][3]
    want = 1.0 / (
        1.0
        + np.exp(
            -np.einsum(
                "nd,d,nd->n", inputs["inputs_row"], rel, inputs["inputs_col"]
            )
        )
    )
    err = np.abs(got - want).max()
    print("max abs err:", err)


# revision 42
# speedup vs baseline: 1.2599x; 1.2599x over previous
"""DistMult decoder kernel for Trainium2 (Bass/Tile), 8-core data-parallel.

Computes sigmoid(einsum('nd,d,nd->n', row, rel, col)) for N=500000, D=256.

Sharding: rows split evenly across 8 cores (62500 each). The selected
relation vector rel = relations[relation_index] is broadcast to [128, 256]
on host (tiny) and replicated to every core.

Per-core layout: the 62500-row shard is viewed as [128, 488, 256]
(n = p*488 + j) plus a 36-row tail, so every DMA moves per-partition
contiguous spans (16 KB for CHUNK=16). Engines:
  - HWDGE (sync + scalar rings): input loads
  - GPSIMD: prod = row * col   (elementwise)
  - DVE: tensor_tensor_reduce: scores[p, j] = sum_d(prod * rel)
  - ACT: sigmoid
"""

import numpy as np

import concourse.bass as bass
import concourse.mybir as mybir
from concourse import tile
from concourse.bass_utils import run_bass_kernel_spmd

N = 500000
D = 256
N_CORES = 8
N_SHARD = N // N_CORES  # 62500
P = 128
J = N_SHARD // P        # 488
MAIN = P * J            # 62464
TAIL = N_SHARD - MAIN   # 36
CHUNK = 16              # j-columns per DMA chunk (16 KB contiguous/partition)

F32 = mybir.dt.float32


def build_program(n_shard: int = N_SHARD, chunk: int = CHUNK, bufs: int = 5) -> bass.Bass:
    p = P
    j_cols = n_shard // p
    main = p * j_cols
    tail = n_shard - main

    nc = bass.Bass()
    row = nc.declare_dram_parameter("row", [n_shard, D], F32, isOutput=False)
    col = nc.declare_dram_parameter("col", [n_shard, D], F32, isOutput=False)
    relb = nc.declare_dram_parameter("relb", [p, D], F32, isOutput=False)
    out = nc.declare_dram_parameter("out", [n_shard], F32, isOutput=True)

    row_m = row[0:main, :].rearrange("(p j) d -> p j d", p=p)
    col_m = col[0:main, :].rearrange("(p j) d -> p j d", p=p)
    out_m = out[0:main].rearrange("(p j) -> p j", p=p)

    mult = mybir.AluOpType.mult
    add = mybir.AluOpType.add
    sig = mybir.ActivationFunctionType.Sigmoid

    # chunk schedule over the main part: small ramp-up chunks first so the
    # DVE starts sooner, then full-size chunks
    sizes = []
    left = j_cols
    for r in (max(1, chunk // 4), max(1, chunk // 2)):
        if left > chunk:
            sizes.append(min(r, left))
            left -= sizes[-1]
    while left > 0:
        sizes.append(min(chunk, left))
        left -= sizes[-1]
    n_chunks = len(sizes)

    from contextlib import ExitStack

    with ExitStack() as es:
        rel_sb = es.enter_context(nc.sbuf_tensor([p, D], F32))
        scores = es.enter_context(nc.sbuf_tensor([p, j_cols + 1], F32))
        rt_buf = es.enter_context(nc.sbuf_tensor([p, bufs * chunk * D], F32))
        ct_buf = es.enter_context(nc.sbuf_tensor([p, bufs * chunk * D], F32))
        rt_t = es.enter_context(nc.sbuf_tensor([p, D], F32))
        ct_t = es.enter_context(nc.sbuf_tensor([p, D], F32))
        rel_sem = es.enter_context(nc.semaphore("rel_sem"))
        row_sems = [
            es.enter_context(nc.semaphore(f"row_sem{b}")) for b in range(bufs)
        ]
        col_sems = [
            es.enter_context(nc.semaphore(f"col_sem{b}")) for b in range(bufs)
        ]
        tt_sems = [
            es.enter_context(nc.semaphore(f"tt_sem{b}")) for b in range(bufs)
        ]
        tt2_sems = [
            es.enter_context(nc.semaphore(f"tt2_sem{b}")) for b in range(bufs)
        ]
        tail_sem = es.enter_context(nc.semaphore("tail_sem"))
        tail_dve_sem = es.enter_context(nc.semaphore("tail_dve_sem"))
        ve_sem = es.enter_context(nc.semaphore("ve_sem"))
        act_sem = es.enter_context(nc.semaphore("act_sem"))
        store_sem = es.enter_context(nc.semaphore("store_sem"))
        block = es.enter_context(nc.Block())

        def rt_slot(c):
            b = c % bufs
            return rt_buf[:, b * chunk * D : (b + 1) * chunk * D]

        def ct_slot(c):
            b = c % bufs
            return ct_buf[:, b * chunk * D : (b + 1) * chunk * D]

        # ve_sem counts STT completions only. Waiting for the cumulative
        # total of all incs issued so far guarantees completion of every one
        # of them, even with unordered retirement. TT completions tick the
        # per-slot tt_sems (one TT in flight per slot at a time, so a
        # cumulative per-slot count is also sound).
        cum_stt = []
        tot = 0
        for k in sizes:
            tot += k
            cum_stt.append(tot)
        stt_total = tot

        # chunk start offsets
        offs = []
        o = 0
        for k in sizes:
            offs.append(o)
            o += k

        # number of store groups (>=128 j-columns each); must match the
        # scalar block's grouping below
        n_store_groups = 0
        acc = 0
        for c, k in enumerate(sizes):
            acc += k
            if acc >= 128 or c == n_chunks - 1:
                n_store_groups += 1
                acc = 0

        @block.sync
        def _(sync):
            col_lead = min(2, bufs - 1)

            def issue_col(c):
                k = sizes[c]
                j0 = offs[c]
                if c >= bufs:
                    # slot reuse: chunk c-bufs must be fully consumed (STTs
                    # are the last readers of both the rt and ct slabs)
                    sync.wait_ge(ve_sem, cum_stt[c - bufs])
                sync.dma_start(
                    ct_slot(c)[:, 0 : k * D], col_m[:, j0 : j0 + k, :]
                ).then_inc(col_sems[c % bufs], 16)

            def issue_row(c):
                k = sizes[c]
                j0 = offs[c]
                # ct-slot gate for chunk c already passed (issued with or
                # before this row's stronger col gate), rt shares thresholds
                sync.dma_start(
                    rt_slot(c)[:, 0 : k * D], row_m[:, j0 : j0 + k, :]
                ).then_inc(row_sems[c % bufs], 16)

            sync.dma_start(rel_sb[:], relb[:]).then_inc(rel_sem, 16)
            if tail:
                # tail loads first: the DVE processes the tail while the
                # first big chunks stream in
                sync.dma_start(rt_t[0:tail, :], row[main:n_shard, :]).then_inc(
                    tail_sem, 16
                )
                sync.dma_start(ct_t[0:tail, :], col[main:n_shard, :]).then_inc(
                    tail_sem, 16
                )
            # cols run col_lead chunks ahead of rows in the FIFO, matching
            # the DVE's TT lookahead
            for c0 in range(min(col_lead, n_chunks)):
                issue_col(c0)
            for c in range(n_chunks):
                if c + col_lead < n_chunks:
                    issue_col(c + col_lead)
                issue_row(c)
            # all stores issued from the scalar engine; just await completion
            sync.wait_ge(store_sem, 16 * (n_store_groups + (1 if tail else 0)))

        @block.vector
        def _(vector):
            def do_tt(c):
                k = sizes[c]
                vector.wait_ge(col_sems[c % bufs], 16 * (c // bufs + 1))
                ct_v = ct_slot(c)[:, 0 : k * D].rearrange("p (k d) -> p k d", d=D)
                vector.tensor_tensor(
                    out=ct_v,
                    in0=ct_v,
                    in1=rel_sb[:].unsqueeze(1).broadcast_to([p, k, D]),
                    op=mult,
                ).then_inc(tt_sems[c % bufs], 1)

            lookahead = min(2, bufs - 1)
            vector.wait_ge(rel_sem, 16)
            if tail:
                # process the 36-row tail first: its loads are issued right
                # after rel, and the DVE would otherwise idle while the first
                # big col chunks stream in
                vector.wait_ge(tail_sem, 32)
                vector.tensor_tensor(
                    out=ct_t[0:tail, :],
                    in0=ct_t[0:tail, :],
                    in1=rel_sb[0:tail, 0:D],
                    op=mult,
                ).then_inc(tail_dve_sem, 1)
                vector.wait_ge(tail_dve_sem, 1)
                vector.scalar_tensor_tensor(
                    out=ct_t[0:tail, :],
                    in0=ct_t[0:tail, :],
                    scalar=1.0,
                    in1=rt_t[0:tail, :],
                    op0=mult,
                    op1=mult,
                    accum_out=scores[0:tail, j_cols : j_cols + 1],
                ).then_inc(tail_dve_sem, 1)
            for c0 in range(min(lookahead, n_chunks)):
                do_tt(c0)
            for c, k in enumerate(sizes):
                j0 = offs[c]
                if c + lookahead < n_chunks:
                    # lookahead: issue later chunks' TTs before this chunk's
                    # STTs so the tt_sem wait below is already satisfied
                    do_tt(c + lookahead)
                vector.wait_ge(tt_sems[c % bufs], c // bufs + 1)
                vector.wait_ge(row_sems[c % bufs], 16 * (c // bufs + 1))
                for jj in range(k):
                    sl = ct_slot(c)[:, jj * D : (jj + 1) * D]
                    vector.scalar_tensor_tensor(
                        out=sl,
                        in0=sl,
                        scalar=1.0,
                        in1=rt_slot(c)[:, jj * D : (jj + 1) * D],
                        op0=mult,
                        op1=mult,
                        accum_out=scores[:, j0 + jj : j0 + jj + 1],
                    ).then_inc(ve_sem, 1)

        # store groups: batch chunks until >=128 j-columns so each store DMA
        # moves >=512 B per partition (avoids the sub-512B RMW penalty)
        groups = []  # (last_chunk_idx, g0, g1)
        g0 = 0
        acc = 0
        for c, k in enumerate(sizes):
            acc += k
            if acc >= 128 or c == n_chunks - 1:
                groups.append((c, g0, g0 + acc))
                g0 += acc
                acc = 0
        n_groups = len(groups)

        @block.scalar
        def _(scalar):
            # grouped sigmoid + store on the ACT HWDGE ring; the tail goes
            # first (its STT runs during ramp-up), groups follow compute
            n_act = 0
            if tail:
                scalar.wait_ge(tail_dve_sem, 2)
                scalar.activation(
                    out=scores[0:tail, j_cols : j_cols + 1],
                    in_=scores[0:tail, j_cols : j_cols + 1],
                    func=sig,
                ).then_inc(act_sem, 1)
                scalar.wait_ge(act_sem, 1)
                n_act = 1
                with nc.allow_non_contiguous_dma(reason="tiny tail store"):
                    scalar.dma_start(
                        out[main:n_shard].rearrange("(p j) -> p j", j=1),
                        scores[0:tail, j_cols : j_cols + 1],
                    ).then_inc(store_sem, 16)
            for gi, (c_last, a, b) in enumerate(groups):
                scalar.wait_ge(ve_sem, cum_stt[c_last])
                scalar.activation(
                    out=scores[:, a:b], in_=scores[:, a:b], func=sig
                ).then_inc(act_sem, 1)
                scalar.wait_ge(act_sem, n_act + gi + 1)
                scalar.dma_start(out_m[:, a:b], scores[:, a:b]).then_inc(
                    store_sem, 16
                )

    return nc


_PROGRAM = None


def _get_program() -> bass.Bass:
    global _PROGRAM
    if _PROGRAM is None:
        _PROGRAM = build_program()
    return _PROGRAM


def make_relb(rel, chunk=CHUNK):
    """Host-side [P, D] broadcast of the selected relation vector."""
    del chunk
    return np.ascontiguousarray(
        np.broadcast_to(np.asarray(rel, np.float32), (P, D))
    )


def _run(inputs_row, inputs_col, relations, relation_index, **spmd_kwargs):
    inputs_row = np.ascontiguousarray(np.asarray(inputs_row, dtype=np.float32))
    inputs_col = np.ascontiguousarray(np.asarray(inputs_col, dtype=np.float32))
    relations = np.asarray(relations, dtype=np.float32)
    idx = int(relation_index)

    relb = make_relb(relations[idx])

    in_maps = []
    for c in range(N_CORES):
        sl = slice(c * N_SHARD, (c + 1) * N_SHARD)
        in_maps.append(
            {
                "row": inputs_row[sl],
                "col": inputs_col[sl],
                "relb": relb,
            }
        )

    nc = _get_program()
    return run_bass_kernel_spmd(nc, in_maps, list(range(N_CORES)), **spmd_kwargs)


def kernel(inputs_row, inputs_col, relations, relation_index):
    results = _run(inputs_row, inputs_col, relations, relation_index).results
    out = np.concatenate([results[c]["out"] for c in range(N_CORES)])
    return out.astype(np.float32, copy=False)


if __name__ == "__main__":
    rng = np.random.default_rng(0)
    inputs = {
        "inputs_row": rng.standard_normal((N, D), dtype=np.float32),
        "inputs_col": rng.standard_normal((N, D), dtype=np.float32),
        "relations": rng.standard_normal((8, D), dtype=np.float32),
        "relation_index": 3,
    }
    got = kernel(**inputs)
    rel = inputs["relations"][3]
    want = 1.0 / (
        1.0
        + np.exp(
            -np.einsum(
                "nd,d,nd->n", inputs["inputs_row"], rel, inputs["inputs_col"]
            )
        )
    )
    err = np.abs(got - want).max()
    print("max abs err:", err)
